# revision 11
# baseline (speedup 1.0000x reference)
"""GCN 2-layer forward on 8 TRN2 NeuronCores (Bass/Tile, SPMD + collectives).

Strategy (hardcoded for N=100000 nodes, E=1.6M edges, 256->64->16 feats):
  - Nodes sharded contiguously: core k owns dst rows [12500k, 12500(k+1)).
  - support1 = emb @ W1 computed on the owning core, AllGathered into a
    per-core full table [100000, 64] in DRAM.
  - spmm (gather + segment_sum) done per 128-dst-node window: edges sorted by
    (window, src-chunk); source rows fetched with dma_gather (int16 indices,
    so sources are split into 4 chunks of 25000 rows); segment-sum expressed
    as one-hot matmuls M^T @ X accumulating in PSUM, where M[e, d] =
    (dst_local[e] == d) * edge_val[e] is built on DVE via iota + tensor_scalar.
  - h = relu(spmm + b1) * dropout, AllGathered, second spmm, then
    out = (A @ h) @ W2 + b2 using associativity of the sparse matmul.
"""
import sys

if "/opt/trn_rl_repo" not in sys.path:
    sys.path.insert(0, "/opt/trn_rl_repo")

import numpy as np

N_NODES = 100000
N_EDGES = 1600000
NFEAT = 256
NHID = 64
NOUT = 16
N_CORES = 8
NPC = N_NODES // N_CORES        # 12500 nodes per core
P = 128
WPC = (NPC + P - 1) // P        # 98 windows per core (last window 84 nodes)
LAST_COLS = NPC - (WPC - 1) * P  # 84
N_CHUNKS = 4
CHUNK = 25000                   # < 32768 so int16 local indices fit
GROUP = 7                       # windows per gather group
NG = WPC // GROUP               # 14 groups
PAD_DST = 999.0                 # one-hot never matches -> zero contribution

_CACHE = {}


def _win_cols(w):
    return LAST_COLS if w == WPC - 1 else P


def _compute_schedule(caps):
    """caps: [WPC, N_CHUNKS] int blocks. Returns block layout shared by host
    packing and device codegen."""
    gc_list = []          # (g, c, block_off, nb)
    win_blocks = [[] for _ in range(WPC)]  # per w: (gc_idx, local_b, global_b)
    slot_base = np.zeros((WPC, N_CHUNKS), np.int64)
    off = 0
    for g in range(NG):
        ws = range(g * GROUP, (g + 1) * GROUP)
        for c in range(N_CHUNKS):
            start = off
            lb = 0
            for w in ws:
                slot_base[w, c] = off * P
                for _ in range(int(caps[w, c])):
                    win_blocks[w].append((len(gc_list), lb, off))
                    lb += 1
                    off += 1
            gc_list.append((g, c, start, off - start))
    return gc_list, win_blocks, slot_base, off  # off == B_tot


def _prepare_host(edge_src, edge_dst, edge_val):
    src = np.asarray(edge_src).astype(np.int64)
    dst = np.asarray(edge_dst).astype(np.int64)
    val = np.asarray(edge_val).astype(np.float32)

    core = dst // NPC
    dloc = dst % NPC
    w = dloc // P
    dst_local = dloc % P
    c = src // CHUNK
    src_local = (src % CHUNK).astype(np.int16)

    # per-core counts per (w, c)
    key_full = ((core * WPC + w) * N_CHUNKS + c).astype(np.int64)
    counts = np.bincount(key_full, minlength=N_CORES * WPC * N_CHUNKS).reshape(
        N_CORES, WPC, N_CHUNKS
    )
    caps = (counts.max(axis=0) + P - 1) // P  # [WPC, N_CHUNKS] blocks
    gc_list, win_blocks, slot_base, B_tot = _compute_schedule(caps)
    S = B_tot * P

    per_core = []
    for k in range(N_CORES):
        m = core == k
        kw, kc = w[m], c[m]
        ksrc, kdst, kval = src_local[m], dst_local[m], val[m]
        key = kw * N_CHUNKS + kc
        order = np.argsort(key, kind="stable")
        key_s = key[order]
        n = key_s.size
        # rank within each (w,c) run
        first = np.zeros(n, np.int64)
        newgrp = np.empty(n, bool)
        newgrp[0] = True
        newgrp[1:] = key_s[1:] != key_s[:-1]
        grp_starts = np.flatnonzero(newgrp)
        first[grp_starts] = grp_starts
        np.maximum.accumulate(first, out=first)
        rank = np.arange(n) - first
        base = slot_base[key_s // N_CHUNKS, key_s % N_CHUNKS]
        pos = base + rank

        idx_slots = np.zeros(S, np.int16)
        dst_slots = np.full(S, PAD_DST, np.float32)
        val_slots = np.zeros(S, np.float32)
        idx_slots[pos] = ksrc[order]
        dst_slots[pos] = kdst[order].astype(np.float32)
        val_slots[pos] = kval[order]

        idx16 = np.tile(idx_slots.reshape(S // 16, 16).T, (8, 1))
        dstloc = np.ascontiguousarray(dst_slots.reshape(B_tot, P).T)
        vals = np.ascontiguousarray(val_slots.reshape(B_tot, P).T)
        per_core.append((np.ascontiguousarray(idx16), dstloc, vals))

    return caps, gc_list, win_blocks, B_tot, per_core


def _build_program(caps, gc_list, win_blocks, B_tot, phases=("support", "ag1", "l1", "ag2", "l2")):
    import concourse.bass as bass
    import concourse.mybir as mybir
    import concourse.tile as tile
    from concourse import bacc
    from concourse.library_config import mlp
    from concourse.masks import make_identity

    dt = mybir.dt
    S16 = B_tot * 8  # idx16 free dim

    nc = bacc.Bacc("TRN2", num_devices=N_CORES)
    embT = nc.dram_tensor("embT", [NFEAT, NPC], dt.float32, kind="ExternalInput")
    W1 = nc.dram_tensor("W1", [NFEAT, NHID], dt.float32, kind="ExternalInput")
    b1r = nc.dram_tensor("b1r", [P, NHID], dt.float32, kind="ExternalInput")
    W2 = nc.dram_tensor("W2", [NHID, NOUT], dt.float32, kind="ExternalInput")
    b2r = nc.dram_tensor("b2r", [P, NOUT], dt.float32, kind="ExternalInput")
    maskd = nc.dram_tensor("maskd", [NPC, NHID], dt.float32, kind="ExternalInput")
    idx16d = nc.dram_tensor("idx16", [P, S16], dt.int16, kind="ExternalInput")
    dstlocd = nc.dram_tensor("dstloc", [P, B_tot], dt.float32, kind="ExternalInput")
    valsd = nc.dram_tensor("vals", [P, B_tot], dt.float32, kind="ExternalInput")
    outd = nc.dram_tensor("out", [NPC, NOUT], dt.float32, kind="ExternalOutput")

    with tile.TileContext(nc) as tc:
        with (
            tc.tile_pool(name="const", bufs=1) as constp,
            tc.tile_pool(name="dram", bufs=1, space="DRAM") as dram,
        ):
            nc.gpsimd.load_library(mlp)

            iota_i = constp.tile([P, P], dt.int32)
            nc.gpsimd.iota(iota_i[:], pattern=[[1, P]], base=0, channel_multiplier=0)
            iota_f = constp.tile([P, P], dt.float32)
            nc.vector.tensor_copy(iota_f[:], iota_i[:])
            ident = constp.tile([P, P], dt.float32)
            make_identity(nc, ident[:])
            w1a = constp.tile([P, NHID], dt.float32)
            w1b = constp.tile([P, NHID], dt.float32)
            nc.sync.dma_start(w1a[:], W1[0:P, :])
            nc.sync.dma_start(w1b[:], W1[P : 2 * P, :])
            w2t = constp.tile([NHID, NOUT], dt.float32)
            nc.sync.dma_start(w2t[:], W2[:])
            b1t = constp.tile([P, NHID], dt.float32)
            nc.sync.dma_start(b1t[:], b1r[:])
            b2t = constp.tile([P, NOUT], dt.float32)
            nc.sync.dma_start(b2t[:], b2r[:])

            ag1_in = dram.tile([NPC, NHID], dt.float32)
            table1 = dram.tile([N_NODES, NHID], dt.float32, addr_space="Shared")
            ag2_in = dram.tile([NPC, NHID], dt.float32)
            table2 = dram.tile([N_NODES, NHID], dt.float32, addr_space="Shared")

            # ---------------- Phase A: support1 = emb @ W1 (own shard) -------
            with tc.tile_pool(name="supp", bufs=2, space="PSUM") as psum_s, \
                 tc.tile_pool(name="supsb", bufs=3) as sup_sb:
                for g in range(NG if "support" in phases else 0):
                    gcol = g * GROUP * P
                    cols_g = min(GROUP * P, NPC - gcol)
                    ea = sup_sb.tile([P, cols_g], dt.float32, tag="ea", bufs=2)
                    eb = sup_sb.tile([P, cols_g], dt.float32, tag="eb", bufs=2)
                    nc.sync.dma_start(ea[:], embT[0:P, gcol : gcol + cols_g])
                    nc.sync.dma_start(eb[:], embT[P : 2 * P, gcol : gcol + cols_g])
                    for wl in range(GROUP):
                        w_ = g * GROUP + wl
                        cols = _win_cols(w_)
                        ps = psum_s.tile([P, NHID], dt.float32, tag="ps", bufs=2,
                                         space="PSUM")
                        nc.tensor.matmul(
                            out=ps[:cols, :],
                            lhsT=ea[:, wl * P : wl * P + cols],
                            rhs=w1a[:], start=True, stop=False)
                        nc.tensor.matmul(
                            out=ps[:cols, :],
                            lhsT=eb[:, wl * P : wl * P + cols],
                            rhs=w1b[:], start=False, stop=True)
                        sup = sup_sb.tile([P, NHID], dt.float32, tag="sup", bufs=3)
                        nc.vector.tensor_copy(sup[:cols, :], ps[:cols, :])
                        nc.sync.dma_start(
                            ag1_in[w_ * P : w_ * P + cols, :], sup[:cols, :])

            if "ag1" in phases:
                nc.gpsimd.collective_compute(
                    "AllGather", mybir.AluOpType.bypass,
                    replica_groups=[list(range(N_CORES))],
                    ins=[ag1_in.opt()], outs=[table1.opt()],
                )

            # ---------------- scatter layers --------------------------------
            dummy = dram.tile([P, NHID], dt.float32)

            def scatter_layer(table, post, do_gather=True, do_compute=True):
                with (
                    tc.tile_pool(name="xsb", bufs=1) as xp,
                    tc.tile_pool(name="meta", bufs=1) as mp,
                    tc.tile_pool(name="mtile", bufs=1) as mt,
                    tc.tile_pool(name="acc", bufs=1, space="PSUM") as accp,
                    tc.tile_pool(name="post", bufs=1) as postp,
                    tc.tile_pool(name="postps", bufs=1, space="PSUM") as postps,
                ):
                    for g in range(NG):
                        ws = list(range(g * GROUP, (g + 1) * GROUP))
                        # group metadata
                        b0 = win_blocks[ws[0]][0][2]
                        bN = win_blocks[ws[-1]][-1][2] + 1
                        nbg = bN - b0
                        dstt = mp.tile([P, nbg], dt.float32, tag="dst", bufs=2)
                        valt = mp.tile([P, nbg], dt.float32, tag="val", bufs=2)
                        nc.sync.dma_start(dstt[:], dstlocd[:, b0:bN])
                        nc.sync.dma_start(valt[:], valsd[:, b0:bN])
                        xt = {}
                        for gi, (gg, c, boff, nb) in enumerate(gc_list):
                            if gg != g or nb == 0:
                                continue
                            x = xp.tile([P, nb, NHID], dt.float32,
                                        tag=f"x{c}", bufs=2)
                            if do_gather:
                                idxs = mp.tile([P, nb * 8], dt.int16,
                                               tag=f"idx{c}", bufs=2)
                                nc.sync.dma_start(
                                    idxs[:], idx16d[:, boff * 8 : (boff + nb) * 8])
                                nc.gpsimd.dma_gather(
                                    x[:], table[c * CHUNK : (c + 1) * CHUNK, :],
                                    idxs[:], nb * P, nb * P, NHID,
                                    single_packet=(nb * P <= 1024))
                                if not do_compute:
                                    nc.sync.dma_start(dummy[:, :], x[:, 0, :])
                            else:
                                nc.vector.memset(x[:, 0, :], 0.0)
                            xt[gi] = x
                        if not do_compute:
                            continue
                        for w_ in ws:
                            acc = accp.tile([P, NHID], dt.float32, tag="acc",
                                            bufs=4, space="PSUM")
                            blocks = win_blocks[w_]
                            for i, (gi, lb, gb) in enumerate(blocks):
                                m = mt.tile([P, P], dt.float32, tag="m", bufs=6)
                                nc.vector.tensor_scalar(
                                    out=m[:], in0=iota_f[:],
                                    scalar1=dstt[:, gb - b0 : gb - b0 + 1],
                                    op0=mybir.AluOpType.is_equal,
                                    scalar2=valt[:, gb - b0 : gb - b0 + 1],
                                    op1=mybir.AluOpType.mult)
                                nc.tensor.matmul(
                                    out=acc[:], lhsT=m[:],
                                    rhs=xt[gi][:, lb, :],
                                    start=(i == 0), stop=(i == len(blocks) - 1))
                            post(w_, acc, postp, postps)

            def post1(w_, acc, postp, postps):
                cols = _win_cols(w_)
                mk = postp.tile([P, NHID], dt.float32, tag="mk", bufs=3)
                nc.sync.dma_start(mk[:cols, :], maskd[w_ * P : w_ * P + cols, :])
                t = postp.tile([P, NHID], dt.float32, tag="t", bufs=3)
                nc.vector.tensor_tensor(
                    out=t[:cols, :], in0=acc[:cols, :], in1=b1t[:cols, :],
                    op=mybir.AluOpType.add)
                t2 = postp.tile([P, NHID], dt.float32, tag="t2", bufs=3)
                nc.vector.tensor_tensor(
                    out=t2[:cols, :], in0=t[:cols, :], in1=mk[:cols, :],
                    op=mybir.AluOpType.mult)
                h = postp.tile([P, NHID], dt.float32, tag="h", bufs=3)
                nc.scalar.activation(
                    out=h[:cols, :], in_=t2[:cols, :],
                    func=mybir.ActivationFunctionType.Relu)
                nc.sync.dma_start(ag2_in[w_ * P : w_ * P + cols, :], h[:cols, :])

            def post2(w_, acc, postp, postps):
                cols = _win_cols(w_)
                gsb = postp.tile([P, NHID], dt.float32, tag="g", bufs=3)
                nc.vector.tensor_copy(gsb[:], acc[:])
                gt_ps = postps.tile([NHID, P], dt.float32, tag="gt", bufs=2,
                                    space="PSUM")
                nc.tensor.transpose(out=gt_ps[:], in_=gsb[:], identity=ident[:])
                gt = postp.tile([NHID, P], dt.float32, tag="gts", bufs=3)
                nc.vector.tensor_copy(gt[:], gt_ps[:])
                ops = postps.tile([P, NOUT], dt.float32, tag="o", bufs=2,
                                  space="PSUM")
                nc.tensor.matmul(out=ops[:], lhsT=gt[:], rhs=w2t[:],
                                 start=True, stop=True)
                o = postp.tile([P, NOUT], dt.float32, tag="ot", bufs=3)
                nc.vector.tensor_tensor(
                    out=o[:cols, :], in0=ops[:cols, :], in1=b2t[:cols, :],
                    op=mybir.AluOpType.add)
                nc.sync.dma_start(outd[w_ * P : w_ * P + cols, :], o[:cols, :])

            if "l1" in phases:
                scatter_layer(table1, post1)
            elif "l1g" in phases:
                scatter_layer(table1, post1, do_gather=True, do_compute=False)
            elif "l1m" in phases:
                scatter_layer(table1, post1, do_gather=False, do_compute=True)
            if "ag2" in phases:
                nc.gpsimd.collective_compute(
                    "AllGather", mybir.AluOpType.bypass,
                    replica_groups=[list(range(N_CORES))],
                    ins=[ag2_in.opt()], outs=[table2.opt()],
                )
            if "l2" in phases:
                scatter_layer(table2, post2)
            else:
                # keep the output defined so the ExternalOutput is written
                with tc.tile_pool(name="dummy", bufs=1) as dp:
                    z = dp.tile([P, NOUT], dt.float32)
                    nc.gpsimd.memset(z[:], 0.0)
                    for w_ in range(WPC):
                        cols = _win_cols(w_)
                        nc.sync.dma_start(outd[w_ * P : w_ * P + cols, :],
                                          z[:cols, :])

    nc.compile()
    return nc


def _run(inputs, trace=False, phases=("support", "ag1", "l1", "ag2", "l2")):
    from concourse.bass_utils import run_bass_kernel_spmd

    embeddings = np.asarray(inputs["embeddings"], np.float32)
    W1 = np.asarray(inputs["W1"], np.float32)
    b1 = np.asarray(inputs["b1"], np.float32)
    W2 = np.asarray(inputs["W2"], np.float32)
    b2 = np.asarray(inputs["b2"], np.float32)
    edge_val = np.asarray(inputs["edge_val"], np.float32)
    dropout_mask = np.asarray(inputs["dropout_mask"], np.float32)
    edge_src = np.asarray(inputs["edge_src"])
    edge_dst = np.asarray(inputs["edge_dst"])

    caps, gc_list, win_blocks, B_tot, per_core = _prepare_host(
        edge_src, edge_dst, edge_val)

    key = (caps.tobytes(), tuple(phases))
    if key not in _CACHE:
        _CACHE[key] = _build_program(caps, gc_list, win_blocks, B_tot,
                                     phases=phases)
    nc = _CACHE[key]

    b1r = np.ascontiguousarray(np.tile(b1[None, :], (P, 1)).astype(np.float32))
    b2r = np.ascontiguousarray(np.tile(b2[None, :], (P, 1)).astype(np.float32))
    in_maps = []
    for k in range(N_CORES):
        idx16, dstloc, vals = per_core[k]
        sl = slice(k * NPC, (k + 1) * NPC)
        in_maps.append({
            "embT": np.ascontiguousarray(embeddings[sl].T),
            "W1": W1, "b1r": b1r, "W2": W2, "b2r": b2r,
            "maskd": np.ascontiguousarray(dropout_mask[sl]),
            "idx16": idx16, "dstloc": dstloc, "vals": vals,
        })

    res = run_bass_kernel_spmd(
        nc, in_maps, core_ids=list(range(N_CORES)), trace=trace)
    out = np.concatenate([res.results[k]["out"] for k in range(N_CORES)], axis=0)
    return out, res


def kernel(**inputs) -> np.ndarray:
    return _run(inputs, trace=False)[0]
